# revision 1
# baseline (speedup 1.0000x reference)
"""Trainium2 Bass kernel for nn_ConsistLoss (retrieval_knn).

Math notes
----------
reference() = mean(|rigid_refine - pred^T|) where
  rigid_refine = rigid_recon - mean_i(laplace_x_i - laplace_y_i)
  laplace_c_i  = (sum_{j in 6NN_c(i)} c_j - 6*q_i) / 5       (c in {x=rigid_recon, y})
The -6*q_i terms cancel in (laplace_x - laplace_y), and only the MEAN over all
i is needed, so:
  mean_vec = ( sum_j cx(j)*x_j - sum_j cy(j)*y_j ) / (5*N)
where cx(j) = #queries having ref j among their 6 nearest (mask column sums).

Device work per core (512 queries x 4096 refs x 2 clouds):
  s[q,j] = 2*q.x_j - |x_j|^2  (= |q|^2 - dist2; row-constant shift is rank-safe)
  computed as one K=4 matmul with lhsT=[q^T; 1], rhs=[2X^T; -|x|^2].
  top-8 per row via DVE InstMax -> threshold t=6th largest; mask = (s >= t);
  column sums of mask via ones-matmul on PE. Host: Kabsch (3x3 SVD) + O(N) tail.
"""

import os
from contextlib import ExitStack

import numpy as np

import concourse.bass as bass  # noqa: F401  (AP types / plumbing)
import concourse.tile as tile
from concourse import bacc, mybir
from concourse.bass_utils import run_bass_kernel_spmd

N = 4096          # points per cloud
NCORES = 8
NQ = N // NCORES  # 512 queries per core
P = 128           # SBUF partitions
QT = NQ // P      # 4 query tiles per core
CHS = 512         # free-dim chunk = one fp32 PSUM bank
CH = N // CHS     # 8 chunks
ACT_CH0 = 4       # chunks >= this use the ACT Sign (+-1) mask path
DVE_COPY_CH = 2   # chunks < this are PSUM->SBUF copied on DVE, rest on ACT
L_K = 6

_cache = {}
last_results = None  # test harness reads exec_time_ns off this


def _build_bass():
    nc = bacc.Bacc(
        "TRN2", target_bir_lowering=False, debug=False, num_devices=NCORES
    )
    f32 = mybir.dt.float32
    bf16 = mybir.dt.bfloat16
    fp8 = mybir.dt.float8e4
    # K=11 bf16 hi/lo split of [2*q ; -|x|^2] dot products (see kernel()):
    # rows 0-2 hiQ*hiX2, 3-5 hiQ*loX2, 6-8 loQ*hiX2, 9 one*(-hi_nx), 10 one*(-lo_nx)
    qa_d = nc.dram_tensor("qa", [11, NQ], bf16, kind="ExternalInput")
    rx_d = nc.dram_tensor("rx", [11, N], bf16, kind="ExternalInput")
    ry_d = nc.dram_tensor("ry", [11, N], bf16, kind="ExternalInput")
    cnt_d = nc.dram_tensor("cnt", [1, 2 * N], f32, kind="ExternalOutput")

    with ExitStack() as ctx:
        tc = ctx.enter_context(tile.TileContext(nc))
        const_pool = ctx.enter_context(tc.tile_pool(name="const", bufs=1))
        s_pool = ctx.enter_context(tc.tile_pool(name="s", bufs=4))
        m_pool = ctx.enter_context(tc.tile_pool(name="m", bufs=2 * QT))
        t8_pool = ctx.enter_context(tc.tile_pool(name="t8", bufs=2 * QT))
        ps_pool = ctx.enter_context(tc.tile_pool(name="ps", bufs=4, space="PSUM"))
        cp_pool = ctx.enter_context(tc.tile_pool(name="cp", bufs=2, space="PSUM"))

        qa = const_pool.tile([11, NQ], bf16)
        nc.sync.dma_start(qa[:], qa_d.ap())
        rx = const_pool.tile([11, N], bf16)
        nc.sync.dma_start(rx[:], rx_d.ap())
        ry = const_pool.tile([11, N], bf16)
        nc.sync.dma_start(ry[:], ry_d.ap())
        ones = const_pool.tile([P, 1], bf16)
        nc.vector.memset(ones[:], 1.0)
        out_sb = const_pool.tile([1, 2 * N], f32)

        for ci, r in enumerate((rx, ry)):
            masks = []
            for qt in range(QT):
                mask = m_pool.tile([P, N], bf16, tag="m", name=f"m{ci}_{qt}")
                masks.append(mask)
                s = s_pool.tile([P, N], f32, tag="s")
                for ch in range(CH):
                    ps = ps_pool.tile([P, CHS], f32, tag="ps")
                    # bf16 split matmul: full-rate (1 cyc/col) on the PE
                    nc.tensor.matmul(
                        ps[:],
                        qa[:, qt * P : (qt + 1) * P],
                        r[:, ch * CHS : (ch + 1) * CHS],
                        start=True,
                        stop=True,
                    )
                    if ch < DVE_COPY_CH:
                        nc.vector.tensor_copy(s[:, ch * CHS : (ch + 1) * CHS], ps[:])
                    else:
                        nc.scalar.copy(s[:, ch * CHS : (ch + 1) * CHS], ps[:])
                t8 = t8_pool.tile([P, 8], f32, tag="t8")
                nc.vector.max(t8[:], s[:])
                # tp_neg = -(t6+t7)/2: strictly-between threshold for Sign
                tp = t8_pool.tile([P, 1], f32, tag="tp")
                nc.vector.tensor_add(tp[:], t8[:, 5:6], t8[:, 6:7])
                nc.vector.tensor_scalar_mul(tp[:], tp[:], -0.5)
                # mask: top-6 of each row. Low chunks on DVE as 0/1 via
                # (s >= t6); high chunks on ACT as -1/+1 via Sign(s - t'),
                # t' strictly between t6 and t7 (host decodes c=(pm+512)/2).
                for ch in range(ACT_CH0):
                    nc.vector.tensor_scalar(
                        mask[:, ch * CHS : (ch + 1) * CHS],
                        s[:, ch * CHS : (ch + 1) * CHS],
                        t8[:, 5:6],
                        None,
                        mybir.AluOpType.is_ge,
                    )
                for ch in range(ACT_CH0, CH):
                    nc.scalar.activation(
                        mask[:, ch * CHS : (ch + 1) * CHS],
                        s[:, ch * CHS : (ch + 1) * CHS],
                        mybir.ActivationFunctionType.Sign,
                        bias=tp[:, 0:1],
                        scale=1.0,
                    )
            # column sums: cnt[ci, j] = #queries of this core with j in their 6NN
            for ch in range(CH):
                cp = cp_pool.tile([1, CHS], f32, tag="cp")
                for qt in range(QT):
                    nc.tensor.matmul(
                        cp[:],
                        ones[:],
                        masks[qt][:, ch * CHS : (ch + 1) * CHS],
                        start=(qt == 0),
                        stop=(qt == QT - 1),
                    )
                if ch < CH // 2:
                    nc.scalar.copy(
                        out_sb[0:1, ci * N + ch * CHS : ci * N + (ch + 1) * CHS],
                        cp[:],
                    )
                else:
                    nc.vector.tensor_copy(
                        out_sb[0:1, ci * N + ch * CHS : ci * N + (ch + 1) * CHS],
                        cp[:],
                    )
        nc.sync.dma_start(cnt_d.ap(), out_sb[:])

    nc.compile()
    return nc


def _get_nc():
    if "nc" not in _cache:
        _cache["nc"] = _build_bass()
    return _cache["nc"]


def _kabsch_recon(input_t, sf_t):
    """Mirror reference's f32 Kabsch pipeline in numpy; returns rigid_recon [N,3]."""
    pc = np.ascontiguousarray(input_t[0].T.astype(np.float32))  # [N,3]
    recon = pc + np.ascontiguousarray(sf_t[0].T.astype(np.float32))
    cp = pc.mean(axis=0)
    cr = recon.mean(axis=0)
    H = (pc - cp).T @ (recon - cr)
    U, _, Vt = np.linalg.svd(H.astype(np.float64))
    d = np.sign(np.linalg.det(Vt.T @ U.T))
    R = Vt.T @ (np.array([1.0, 1.0, d])[:, None] * U.T)
    t = cr.astype(np.float64) - R @ cp.astype(np.float64)
    return (pc.astype(np.float64) @ R.T + t).astype(np.float32)


def kernel(input_t, sf_t, y1, pred):
    input_t = np.asarray(input_t, dtype=np.float32)
    sf_t = np.asarray(sf_t, dtype=np.float32)
    y1 = np.asarray(y1, dtype=np.float32)
    pred = np.asarray(pred, dtype=np.float32)

    X = _kabsch_recon(input_t, sf_t)                       # rigid_recon [N,3]
    Y = np.ascontiguousarray(y1[0].T.astype(np.float32))   # [N,3]

    import ml_dtypes

    bf = ml_dtypes.bfloat16

    def _split_ref(R):
        # rhs rows for s = 2*q.r - |r|^2 via bf16 hi/lo products
        R2 = (2.0 * R).astype(np.float32)                  # [N,3]
        hiR = R2.astype(bf)
        loR = (R2 - hiR.astype(np.float32)).astype(bf)
        nr = (R.astype(np.float32) ** 2).sum(axis=1, dtype=np.float32)
        hin = nr.astype(bf)
        lon = (nr - hin.astype(np.float32)).astype(bf)
        return np.ascontiguousarray(
            np.concatenate(
                [hiR.T, loR.T, hiR.T, -hin[None, :], -lon[None, :]], axis=0
            ).astype(bf)
        )  # [11, N]

    rx = _split_ref(X)
    ry = _split_ref(Y)

    in_maps = []
    for c in range(NCORES):
        q = X[c * NQ : (c + 1) * NQ].astype(np.float32)    # [NQ,3]
        hiQ = q.astype(bf)
        loQ = (q - hiQ.astype(np.float32)).astype(bf)
        one = np.ones((1, NQ), np.float32).astype(bf)
        qa = np.ascontiguousarray(
            np.concatenate([hiQ.T, hiQ.T, loQ.T, one, one], axis=0).astype(bf)
        )  # [11, NQ]
        in_maps.append({"qa": qa, "rx": rx, "ry": ry})

    nc = _get_nc()
    global last_results
    res = run_bass_kernel_spmd(nc, in_maps, core_ids=list(range(NCORES)))
    last_results = res

    cnt = np.stack([r["cnt"].reshape(2, N) for r in res.results])  # [8, 2, N]
    cnt = cnt.astype(np.float64)
    # chunks >= ACT_CH0 hold +-1 sums over NQ rows: c = (pm + NQ) / 2
    cnt[:, :, ACT_CH0 * CHS :] = (cnt[:, :, ACT_CH0 * CHS :] + NQ) / 2.0
    cx = cnt[:, 0, :].sum(axis=0)
    cy = cnt[:, 1, :].sum(axis=0)

    Sx = X.astype(np.float64).T @ cx                       # [3]
    Sy = Y.astype(np.float64).T @ cy
    mean_vec = ((Sx - Sy) / ((L_K - 1) * N)).astype(np.float32)

    rigid_refine = X - mean_vec[None, :]
    predT = np.ascontiguousarray(pred[0].T.astype(np.float32))
    loss = np.abs(rigid_refine.astype(np.float64) - predT.astype(np.float64)).mean()
    return np.float32(loss)



# revision 9
# speedup vs baseline: 1.1000x; 1.1000x over previous
"""Trainium2 Bass kernel for nn_ConsistLoss (retrieval_knn).

Math notes
----------
reference() = mean(|rigid_refine - pred^T|) where
  rigid_refine = rigid_recon - mean_i(laplace_x_i - laplace_y_i)
  laplace_c_i  = (sum_{j in 6NN_c(i)} c_j - 6*q_i) / 5       (c in {x=rigid_recon, y})
The -6*q_i terms cancel in (laplace_x - laplace_y), and only the MEAN over all
i is needed, so:
  mean_vec = ( sum_j cx(j)*x_j - sum_j cy(j)*y_j ) / (5*N)
where cx(j) = #queries having ref j among their 6 nearest (mask column sums).

Device work per core (512 queries x 4096 refs x 2 clouds):
  s[q,j] = 2*q.x_j - |x_j|^2  (row-constant |q|^2 shift is rank-safe), computed
  as fp8e4m3 DoubleRow matmuls (14 split-product rows in 7 partition pairs,
  0.5 cyc/col).  PSUM f32 -> SBUF fp16 copy (ACT), then DVE: pairwise
  tensor-max folds 4096->512, MAX8 -> t6/t7 thresholds.  Masks: low ref half
  as is_ge(s16, t6) -> bf16 on DVE (4x mode); high half as Sign(s16 - mid)
  -> fp8 +-1 on ACT.  Column sums on PE: bf16 ones-matmul per qt for the low
  half, fp8 DoubleRow pair-contraction (2 query tiles per matmul) for the
  high half.  Host: Kabsch (3x3 SVD) + O(N) decode/reductions.
"""

import os
from contextlib import ExitStack

import numpy as np

import concourse.bass as bass  # noqa: F401  (AP types / plumbing)
import concourse.tile as tile
from concourse import bacc, mybir
from concourse.bass_utils import run_bass_kernel_spmd

N = 4096          # points per cloud
NCORES = 8
NQ = N // NCORES  # 512 queries per core
P = 128           # SBUF partitions
QT = NQ // P      # 4 query tiles per core
CHS = 512         # fp32 PSUM bank = 512 f32
KP = 7            # fp8 split rows: 14 = 7 partitions x 2 DoubleRow pairs
NB2 = N // 2      # ref split: low half bf16 masks (DVE), high half fp8 (ACT)
L_K = 6

_cache = {}
last_results = None  # test harness reads exec_time_ns off this


def _build_bass():
    nc = bacc.Bacc(
        "TRN2", target_bir_lowering=False, debug=False, num_devices=NCORES
    )
    f32 = mybir.dt.float32
    f16 = mybir.dt.float16
    bf16 = mybir.dt.bfloat16
    fp8 = mybir.dt.float8e4
    DR = mybir.MatmulPerfMode.DoubleRow

    # 14 fp8 rows of the score contraction (plain fp8 matmul; 128-col weights
    # trigger the compiler's Fast Weight Load, so no DoubleRow needed here)
    # s = 2*q.r - |r|^2: rows = {q1,q2} x {r1,r2} per dim (12) + 1*(-n1), 1*(-n2)
    qa_d = nc.dram_tensor("qa", [2 * KP, NQ], fp8, kind="ExternalInput")
    rx_d = nc.dram_tensor("rx", [2 * KP, N], fp8, kind="ExternalInput")
    ry_d = nc.dram_tensor("ry", [2 * KP, N], fp8, kind="ExternalInput")
    cnt_d = nc.dram_tensor("cnt", [1, 2 * N], f32, kind="ExternalOutput")

    with ExitStack() as ctx:
        tc = ctx.enter_context(tile.TileContext(nc))
        const_pool = ctx.enter_context(tc.tile_pool(name="const", bufs=1))
        s_pool = ctx.enter_context(tc.tile_pool(name="s16", bufs=2))
        f_pool = ctx.enter_context(tc.tile_pool(name="fold", bufs=2))
        t_pool = ctx.enter_context(tc.tile_pool(name="t8", bufs=2))
        mb_pool = ctx.enter_context(tc.tile_pool(name="mb", bufs=2 * QT))
        mf_pool = ctx.enter_context(tc.tile_pool(name="mf", bufs=2 * (QT // 2)))
        ps_pool = ctx.enter_context(tc.tile_pool(name="ps", bufs=3, space="PSUM"))
        cp_pool = ctx.enter_context(tc.tile_pool(name="cp", bufs=2, space="PSUM"))

        qa = const_pool.tile([2 * KP, NQ], fp8)
        nc.sync.dma_start(qa[:], qa_d.ap())
        rx = const_pool.tile([2 * KP, N], fp8)
        nc.sync.dma_start(rx[:], rx_d.ap())
        ry = const_pool.tile([2 * KP, N], fp8)
        nc.sync.dma_start(ry[:], ry_d.ap())
        # DoubleRow ldweights needs the pair-dim step to be 16B-aligned, so
        # allocate [P, 2, 16] and slice the first column.
        ones8 = const_pool.tile([P, 2, 16], fp8)
        nc.vector.memset(ones8[:], 1.0)
        onesb = const_pool.tile([P, 1], bf16)
        nc.vector.memset(onesb[:], 1.0)
        out_sb = const_pool.tile([1, 2 * N], f32)

        for ci, r in enumerate((rx, ry)):
            mbs = []   # bf16 masks per qt (refs 0..NB2)
            mfs = []   # fp8 +-1 mask pair tiles (refs NB2..N), one per qt pair
            for qt in range(QT):
                s16 = s_pool.tile([P, N], f16, tag="s16")
                for h in range(4):  # 4 PSUM tiles of [P, 1024] (2 banks each)
                    ps = ps_pool.tile([P, 2 * CHS], f32, tag="ps")
                    for k in range(2):
                        nc.tensor.matmul(
                            ps[:, k * CHS : (k + 1) * CHS],
                            qa[:, qt * P : (qt + 1) * P],
                            r[:, (2 * h + k) * CHS : (2 * h + k + 1) * CHS],
                            start=True,
                            stop=True,
                        )
                    # PSUM f32 -> SBUF fp16 (ACT sits closer to PSUM; DVE
                    # takes one of the four to balance)
                    dst = s16[:, h * 2 * CHS : (h + 1) * 2 * CHS]
                    if h == 3:
                        nc.vector.tensor_copy(dst, ps[:])
                    else:
                        nc.scalar.copy(dst, ps[:])
                # threshold: fold 4096->512 by pairwise max, then top-8
                f1 = f_pool.tile([P, N // 2], f16, tag="f1")
                nc.vector.tensor_max(f1[:], s16[:, : N // 2], s16[:, N // 2 :])
                f2 = f_pool.tile([P, N // 4], f16, tag="f2")
                nc.vector.tensor_max(f2[:], f1[:, : N // 4], f1[:, N // 4 :])
                f3 = f_pool.tile([P, N // 8], f16, tag="f3")
                nc.vector.tensor_max(f3[:], f2[:, : N // 8], f2[:, N // 8 :])
                t8 = t_pool.tile([P, 8], f32, tag="t8")
                nc.vector.max(t8[:], f3[:])
                # Sign path needs a strictly-between threshold: -(t6+t7)/2
                tn = t_pool.tile([P, 1], f32, tag="tn")
                nc.vector.tensor_add(tn[:], t8[:, 5:6], t8[:, 6:7])
                nc.vector.tensor_scalar_mul(tn[:], tn[:], -0.5)
                # masks: refs [0, NB2) as 0/1 bf16 on DVE (4x tensor_scalar);
                # refs [NB2, N) as +-1 fp8 on ACT (Sign, bias per partition)
                mb = mb_pool.tile([P, NB2], bf16, tag="mb", name=f"mb{ci}_{qt}")
                mbs.append(mb)
                for u in range(2):
                    nc.vector.tensor_scalar(
                        mb[:, u * 1024 : (u + 1) * 1024],
                        s16[:, u * 1024 : (u + 1) * 1024],
                        t8[:, 5:6],
                        None,
                        mybir.AluOpType.is_ge,
                    )
                if qt % 2 == 0:
                    mf = mf_pool.tile(
                        [P, 2, NB2], fp8, tag="mf", name=f"mf{ci}_{qt // 2}"
                    )
                    mfs.append(mf)
                else:
                    mf = mfs[-1]
                qi = qt % 2
                for u in range(2):
                    nc.scalar.activation(
                        mf[:, qi : qi + 1, u * 1024 : (u + 1) * 1024],
                        s16[:, NB2 + u * 1024 : NB2 + (u + 1) * 1024],
                        mybir.ActivationFunctionType.Sign,
                        bias=tn[:, 0:1],
                        scale=1.0,
                    )
            # column sums. Low half: bf16 ones-matmul, 4 qt accumulated.
            for ch in range(NB2 // CHS):
                cp = cp_pool.tile([1, CHS], f32, tag="cp")
                for qt in range(QT):
                    nc.tensor.matmul(
                        cp[:],
                        onesb[:],
                        mbs[qt][:, ch * CHS : (ch + 1) * CHS],
                        start=(qt == 0),
                        stop=(qt == QT - 1),
                    )
                if ch % 2 == 0:
                    nc.scalar.copy(
                        out_sb[0:1, ci * N + ch * CHS : ci * N + (ch + 1) * CHS],
                        cp[:],
                    )
                else:
                    nc.vector.tensor_copy(
                        out_sb[0:1, ci * N + ch * CHS : ci * N + (ch + 1) * CHS],
                        cp[:],
                    )
            # High half: fp8 DoubleRow, 2 qt per matmul, 2 pair-tiles accum.
            for ch in range(NB2 // CHS):
                cp = cp_pool.tile([1, CHS], f32, tag="cp")
                for pi in range(QT // 2):
                    nc.tensor.matmul(
                        cp[:],
                        ones8[:, :, 0:1],
                        mfs[pi][:, :, ch * CHS : (ch + 1) * CHS],
                        start=(pi == 0),
                        stop=(pi == QT // 2 - 1),
                        perf_mode=DR,
                    )
                base = ci * N + NB2 + ch * CHS
                if ch % 2 == 0:
                    nc.vector.tensor_copy(out_sb[0:1, base : base + CHS], cp[:])
                else:
                    nc.scalar.copy(out_sb[0:1, base : base + CHS], cp[:])
        nc.sync.dma_start(cnt_d.ap(), out_sb[:])

    nc.compile()
    return nc


def _get_nc():
    if "nc" not in _cache:
        _cache["nc"] = _build_bass()
    return _cache["nc"]


def _kabsch_recon(input_t, sf_t):
    """Mirror reference's f32 Kabsch pipeline in numpy; returns rigid_recon [N,3]."""
    pc = np.ascontiguousarray(input_t[0].T.astype(np.float32))  # [N,3]
    recon = pc + np.ascontiguousarray(sf_t[0].T.astype(np.float32))
    cp = pc.mean(axis=0)
    cr = recon.mean(axis=0)
    H = (pc - cp).T @ (recon - cr)
    U, _, Vt = np.linalg.svd(H.astype(np.float64))
    d = np.sign(np.linalg.det(Vt.T @ U.T))
    R = Vt.T @ (np.array([1.0, 1.0, d])[:, None] * U.T)
    t = cr.astype(np.float64) - R @ cp.astype(np.float64)
    return (pc.astype(np.float64) @ R.T + t).astype(np.float32)


def kernel(input_t, sf_t, y1, pred):
    input_t = np.asarray(input_t, dtype=np.float32)
    sf_t = np.asarray(sf_t, dtype=np.float32)
    y1 = np.asarray(y1, dtype=np.float32)
    pred = np.asarray(pred, dtype=np.float32)

    X = _kabsch_recon(input_t, sf_t)                       # rigid_recon [N,3]
    Y = np.ascontiguousarray(y1[0].T.astype(np.float32))   # [N,3]

    f8np = mybir.dt.np(mybir.dt.float8e4)

    def _split8(v, terms):
        out = []
        res = v.astype(np.float32)
        for _ in range(terms):
            h = res.astype(f8np)
            out.append(h)
            res = (res - h.astype(np.float32)).astype(np.float32)
        return out

    def _pack_ref(R):
        # rhs rows r=0..13 -> [7, 2, N]: (p, i) = (r//2, r%2)
        R2 = (2.0 * R).astype(np.float32)                  # [N,3]
        r1, r2 = _split8(R2, 2)                            # [N,3] fp8 each
        nr = (R.astype(np.float32) ** 2).sum(axis=1, dtype=np.float32)
        n1, n2 = _split8(nr, 2)
        rows = [r1.T[d] for d in range(3)] + [r2.T[d] for d in range(3)]
        rows += [r1.T[d] for d in range(3)] + [r2.T[d] for d in range(3)]
        rows += [-n1, -n2]
        return np.ascontiguousarray(np.stack(rows).astype(f8np))

    rx = _pack_ref(X)
    ry = _pack_ref(Y)

    in_maps = []
    one = np.ones(NQ, np.float32).astype(f8np)
    for c in range(NCORES):
        q = X[c * NQ : (c + 1) * NQ].astype(np.float32)    # [NQ,3]
        q1, q2 = _split8(q, 2)
        rows = [q1.T[d] for d in range(3)] * 2 + [q2.T[d] for d in range(3)] * 2
        rows += [one, one]
        qa = np.ascontiguousarray(np.stack(rows).astype(f8np))
        in_maps.append({"qa": qa, "rx": rx, "ry": ry})

    nc = _get_nc()
    global last_results
    res = run_bass_kernel_spmd(nc, in_maps, core_ids=list(range(NCORES)))
    last_results = res

    cnt = np.stack([r["cnt"].reshape(2, N) for r in res.results])  # [8, 2, N]
    cnt = cnt.astype(np.float64)
    # high ref half holds +-1 sums over NQ rows: c = (pm + NQ) / 2
    cnt[:, :, NB2:] = (cnt[:, :, NB2:] + NQ) / 2.0
    cx = cnt[:, 0, :].sum(axis=0)
    cy = cnt[:, 1, :].sum(axis=0)

    Sx = X.astype(np.float64).T @ cx                       # [3]
    Sy = Y.astype(np.float64).T @ cy
    mean_vec = ((Sx - Sy) / ((L_K - 1) * N)).astype(np.float32)

    rigid_refine = X - mean_vec[None, :]
    predT = np.ascontiguousarray(pred[0].T.astype(np.float32))
    loss = np.abs(rigid_refine.astype(np.float64) - predT.astype(np.float64)).mean()
    return np.float32(loss)


# revision 26
# speedup vs baseline: 1.2362x; 1.1238x over previous
"""Trainium2 Bass kernel for nn_ConsistLoss (retrieval_knn).

Math notes
----------
reference() = mean(|rigid_refine - pred^T|) where
  rigid_refine = rigid_recon - mean_i(laplace_x_i - laplace_y_i)
  laplace_c_i  = (sum_{j in 6NN_c(i)} c_j - 6*q_i) / 5       (c in {x=rigid_recon, y})
The -6*q_i terms cancel in (laplace_x - laplace_y), and only the MEAN over all
i is needed, so:
  mean_vec = ( sum_j cx(j)*x_j - sum_j cy(j)*y_j ) / (5*N)
where cx(j) = #queries having ref j among their 6 nearest (mask column sums).

Device work per core (512 queries x 4096 refs x 2 clouds):
  s[q,j] = 2*q.x_j - |x_j|^2  (row-constant |q|^2 shift is rank-safe), via
  fp8e4m3 DoubleRow matmuls: 14 split-product rows in 7 partition pairs,
  0.5 cyc/col.  PSUM f32 -> SBUF fp16 copies (ACT 3 / DVE 1 per query tile),
  threshold via pairwise tensor-max folds 4096->512 (fold1 DVE, fold2/3
  GpSimd) + MAX8.  Masks: refs [0,3072) is_ge(s16,t6) -> bf16 on DVE (4x);
  refs [3072,4096) Sign(s16-mid) -> fp8 +-1 on ACT.  Column sums on PE
  incrementally per query tile (bf16 ones-matmul; fp8 DoubleRow contracts 2
  query tiles per matmul), accumulated in PSUM at 4 partition offsets so one
  [4,512] copy drains 4 chunk-sums.  Host: Kabsch (3x3 SVD) + O(N) decode.
"""

import os
from contextlib import ExitStack

import numpy as np

import concourse.bass as bass  # noqa: F401  (AP types / plumbing)
import concourse.tile as tile
from concourse import bacc, mybir
from concourse.bass_utils import run_bass_kernel_spmd

N = 4096          # points per cloud
NCORES = 8
NQ = N // NCORES  # 512 queries per core
P = 128           # SBUF partitions
QT = NQ // P      # 4 query tiles per core
CHS = 512         # fp32 PSUM bank = 512 f32
KP = 7            # fp8 split rows: 14 = 7 partitions x 2 DoubleRow pairs
NBF = 3072        # refs [0, NBF) -> bf16 masks (DVE), rest fp8 Sign (ACT)
L_K = 6

_cache = {}
last_results = None  # test harness reads exec_time_ns off this


def _build_bass():
    nc = bacc.Bacc(
        "TRN2", target_bir_lowering=False, debug=False, num_devices=NCORES
    )
    f32 = mybir.dt.float32
    f16 = mybir.dt.float16
    bf16 = mybir.dt.bfloat16
    fp8 = mybir.dt.float8e4
    DR = mybir.MatmulPerfMode.DoubleRow
    NF8 = N - NBF

    qa_d = nc.dram_tensor("qa", [KP, 2, NQ], fp8, kind="ExternalInput")
    rx_d = nc.dram_tensor("rx", [KP, 2, N], fp8, kind="ExternalInput")
    ry_d = nc.dram_tensor("ry", [KP, 2, N], fp8, kind="ExternalInput")
    cnt_d = nc.dram_tensor("cnt", [1, 2 * N], f32, kind="ExternalOutput")

    with ExitStack() as ctx:
        tc = ctx.enter_context(tile.TileContext(nc))
        const_pool = ctx.enter_context(tc.tile_pool(name="const", bufs=1))
        s_pool = ctx.enter_context(tc.tile_pool(name="s16", bufs=3))
        f_pool = ctx.enter_context(tc.tile_pool(name="fold", bufs=3))
        t_pool = ctx.enter_context(tc.tile_pool(name="t8", bufs=3))
        mb_pool = ctx.enter_context(tc.tile_pool(name="mb", bufs=6))
        mf_pool = ctx.enter_context(tc.tile_pool(name="mf", bufs=4))
        ps_pool = ctx.enter_context(tc.tile_pool(name="ps", bufs=2, space="PSUM"))
        cp_pool = ctx.enter_context(tc.tile_pool(name="cp", bufs=2, space="PSUM"))

        qa = const_pool.tile([KP, 2, NQ], fp8)
        nc.sync.dma_start(qa[:], qa_d.ap())
        rx = const_pool.tile([KP, 2, N], fp8)
        nc.sync.dma_start(rx[:], rx_d.ap())
        ry = const_pool.tile([KP, 2, N], fp8)
        nc.sync.dma_start(ry[:], ry_d.ap())
        # DoubleRow ldweights needs the pair-dim step 16B-aligned -> pad
        ones8 = const_pool.tile([P, 2, 16], fp8)
        nc.vector.memset(ones8[:], 1.0)
        onesb = const_pool.tile([P, 1], bf16)
        nc.vector.memset(onesb[:], 1.0)
        out_sb = const_pool.tile([1, 2 * N], f32)

        # --- software-pipelined main loop ------------------------------
        # iteration T: PE scores(T), ACT copies(T); DVE threshold+masks(T-1);
        # PE colsum chunks as clouds complete.  Global tile index T = 4*ci+qt.
        NT = 2 * QT
        state = {}      # T -> (s16, mb, mf, t8)
        mfs = {}        # ci -> [pair0, pair1]
        pend = []       # colsum chunk-sum work items (ci, ch)
        ndrain = [0]

        def emit_scores(T):
            ci, qt = divmod(T, QT)
            r = rx if ci == 0 else ry
            s16 = s_pool.tile([P, N], f16, tag="s16", name=f"s16_{T}")
            for h in range(4):  # 4 PSUM tiles of [P, 1024] (2 banks each)
                ps = ps_pool.tile([P, 2 * CHS], f32, tag="ps", name=f"ps{T}_{h}")
                for k in range(2):
                    nc.tensor.matmul(
                        ps[:, k * CHS : (k + 1) * CHS],
                        qa[:, :, qt * P : (qt + 1) * P],
                        r[:, :, (2 * h + k) * CHS : (2 * h + k + 1) * CHS],
                        start=True,
                        stop=True,
                        perf_mode=DR,
                    )
                nc.scalar.copy(s16[:, h * 2 * CHS : (h + 1) * 2 * CHS], ps[:])
            state[T] = s16

        def emit_threshold_masks(T):
            ci, qt = divmod(T, QT)
            s16 = state[T]
            f1 = f_pool.tile([P, N // 2], f16, tag="f1", name=f"f1_{T}")
            nc.vector.tensor_max(f1[:], s16[:, : N // 2], s16[:, N // 2 :])
            f2 = f_pool.tile([P, N // 4], f16, tag="f2", name=f"f2_{T}")
            nc.vector.tensor_max(f2[:], f1[:, : N // 4], f1[:, N // 4 :])
            f3 = f_pool.tile([P, N // 8], f16, tag="f3", name=f"f3_{T}")
            nc.vector.tensor_max(f3[:], f2[:, : N // 8], f2[:, N // 8 :])
            t8 = t_pool.tile([P, 8], f32, tag="t8", name=f"t8_{T}")
            nc.vector.max(t8[:], f3[:])
            # masks vs t6 (exact top-6 incl. fp16 ties):
            # refs [0, NBF): 0/1 bf16 (4x); refs [NBF, N): 0/1 fp8 (2x)
            mb = mb_pool.tile([P, NBF], bf16, tag="mb", name=f"mb_{T}")
            nc.vector.tensor_scalar(
                mb[:], s16[:, :NBF], t8[:, 5:6], None, mybir.AluOpType.is_ge
            )
            if qt % 2 == 0:
                mfs.setdefault(ci, []).append(
                    mf_pool.tile([P, 2, NF8], fp8, tag="mf", name=f"mf_{T}")
                )
            mf = mfs[ci][qt // 2]
            qi = qt % 2
            nc.vector.tensor_scalar(
                mf[:, qi : qi + 1, :],
                s16[:, NBF:N],
                t8[:, 5:6],
                None,
                mybir.AluOpType.is_ge,
            )
            state[T] = (s16, mb)
            if qt == QT - 1:
                pend.extend((ci, ch) for ch in range(N // CHS))

        mbq = {}  # (ci, qt) -> mb tile

        def emit_colsum(budget):
            # emit up to `budget` chunk-sums; drain cp tiles pairwise
            k = 0
            while pend and (budget is None or k < budget):
                ci, ch = pend.pop(0)
                t16 = ci * 8 + ch
                sub = t16 % 2
                if sub == 0:
                    cps = cp_pool.tile(
                        [1, 2 * CHS], f32, tag="cp", name=f"cp_{t16}"
                    )
                    state[("cp", ci, ch // 2)] = cps
                else:
                    cps = state[("cp", ci, ch // 2)]
                dst = cps[:, sub * CHS : (sub + 1) * CHS]
                if ch < NBF // CHS:
                    for qt in range(QT):
                        nc.tensor.matmul(
                            dst,
                            onesb[:],
                            mbq[(ci, qt)][:, ch * CHS : (ch + 1) * CHS],
                            start=(qt == 0),
                            stop=(qt == QT - 1),
                        )
                else:
                    chf = ch - NBF // CHS
                    for pi in range(2):
                        nc.tensor.matmul(
                            dst,
                            ones8[:, :, 0:1],
                            mfs[ci][pi][:, :, chf * CHS : (chf + 1) * CHS],
                            start=(pi == 0),
                            stop=(pi == 1),
                            perf_mode=DR,
                        )
                if sub == 1:
                    col = ci * N + (ch - 1) * CHS
                    dstc = out_sb[0:1, col : col + 2 * CHS]
                    if ndrain[0] % 2 == 0:
                        nc.scalar.copy(dstc, cps[:])
                    else:
                        nc.vector.tensor_copy(dstc, cps[:])
                    ndrain[0] += 1
                k += 1

        for T in range(NT + 1):
            if T < NT:
                emit_scores(T)
            if T >= 1:
                emit_threshold_masks(T - 1)
                ci, qt = divmod(T - 1, QT)
                mbq[(ci, qt)] = state[T - 1][1]
            emit_colsum(3 if T < NT else None)
        emit_colsum(None)
        nc.sync.dma_start(cnt_d.ap(), out_sb[:])

    nc.compile()
    return nc


def _get_nc():
    if "nc" not in _cache:
        _cache["nc"] = _build_bass()
    return _cache["nc"]


def _kabsch_recon(input_t, sf_t):
    """Mirror reference's f32 Kabsch pipeline in numpy; returns rigid_recon [N,3]."""
    pc = np.ascontiguousarray(input_t[0].T.astype(np.float32))  # [N,3]
    recon = pc + np.ascontiguousarray(sf_t[0].T.astype(np.float32))
    cp = pc.mean(axis=0)
    cr = recon.mean(axis=0)
    H = (pc - cp).T @ (recon - cr)
    U, _, Vt = np.linalg.svd(H.astype(np.float64))
    d = np.sign(np.linalg.det(Vt.T @ U.T))
    R = Vt.T @ (np.array([1.0, 1.0, d])[:, None] * U.T)
    t = cr.astype(np.float64) - R @ cp.astype(np.float64)
    return (pc.astype(np.float64) @ R.T + t).astype(np.float32)


def _split8(v, terms, f8np):
    out = []
    res = v.astype(np.float32)
    for _ in range(terms):
        h = res.astype(f8np)
        out.append(h)
        res = (res - h.astype(np.float32)).astype(np.float32)
    return out


def kernel(input_t, sf_t, y1, pred):
    input_t = np.asarray(input_t, dtype=np.float32)
    sf_t = np.asarray(sf_t, dtype=np.float32)
    y1 = np.asarray(y1, dtype=np.float32)
    pred = np.asarray(pred, dtype=np.float32)

    X = _kabsch_recon(input_t, sf_t)                       # rigid_recon [N,3]
    Y = np.ascontiguousarray(y1[0].T.astype(np.float32))   # [N,3]

    f8np = mybir.dt.np(mybir.dt.float8e4)

    def _pack_ref(R):
        # rows r=0..13 -> [7, 2, N]: (p, i) = (r//2, r%2)
        R2 = (2.0 * R).astype(np.float32)                  # [N,3]
        r1, r2 = _split8(R2, 2, f8np)                      # [N,3] fp8 each
        nr = (R.astype(np.float32) ** 2).sum(axis=1, dtype=np.float32)
        n1, n2 = _split8(nr, 2, f8np)
        rows = [r1.T[d] for d in range(3)] + [r2.T[d] for d in range(3)]
        rows += [r1.T[d] for d in range(3)] + [r2.T[d] for d in range(3)]
        rows += [-n1, -n2]
        return np.ascontiguousarray(np.stack(rows).astype(f8np).reshape(KP, 2, N))

    rx = _pack_ref(X)
    ry = _pack_ref(Y)

    in_maps = []
    one = np.ones(NQ, np.float32).astype(f8np)
    for c in range(NCORES):
        q = X[c * NQ : (c + 1) * NQ].astype(np.float32)    # [NQ,3]
        q1, q2 = _split8(q, 2, f8np)
        rows = [q1.T[d] for d in range(3)] * 2 + [q2.T[d] for d in range(3)] * 2
        rows += [one, one]
        qa = np.ascontiguousarray(np.stack(rows).astype(f8np).reshape(KP, 2, NQ))
        in_maps.append({"qa": qa, "rx": rx, "ry": ry})

    nc = _get_nc()
    global last_results
    res = run_bass_kernel_spmd(nc, in_maps, core_ids=list(range(NCORES)))
    last_results = res

    cnt = np.stack([r["cnt"].reshape(2, N) for r in res.results]).astype(np.float64)
    cx = cnt[:, 0, :].sum(axis=0)
    cy = cnt[:, 1, :].sum(axis=0)

    Sx = X.astype(np.float64).T @ cx                       # [3]
    Sy = Y.astype(np.float64).T @ cy
    mean_vec = ((Sx - Sy) / ((L_K - 1) * N)).astype(np.float32)

    rigid_refine = X - mean_vec[None, :]
    predT = np.ascontiguousarray(pred[0].T.astype(np.float32))
    loss = np.abs(rigid_refine.astype(np.float64) - predT.astype(np.float64)).mean()
    return np.float32(loss)
